# revision 20
# baseline (speedup 1.0000x reference)
"""Trainium2 Bass kernel for DissipativeSimplestRINN.

Recurrent implicit NN: per time step, a warm-started tanh fixed-point solve
feeds an explicit-Euler integration of a small linear plant.  B=1024 batch is
sharded 8 ways (128/core); each core runs its batch slice through all T=1024
steps.

Numerical scheme (validated against the full reference in numpy/bf16 at
rel err 5.7e-3; gate is 2e-2):
  carry chain (3 links/step, on the critical path):
      w1 = tanh(eb_t + DVWB c_{t-1});  w2 = tanh(eb_t + DVWB w1);
      c_t = tanh(eb_t + DVWB w2)
  where DVWB = Dvw + DT*Bw@Cv and eb_t is the bias expansion over
  [x_{t-1}; y_{t-1}; y_t] (so the chain never waits on the x update).
  u-track (2 extra refinements, exact bias, off the critical path):
      wx_t = tanh(bx_t + Dvw c_t);  wxx_t = tanh(bx_t + Dvw wx_t)
      u_t = Cu x_t + Duw wxx_t + Duy y_t
  with bx_t = Cv x_t + Dvy y_t from the fp32 PSUM x accumulator.  wx/wxx of
  step t execute during step t+1, filling the ACT gaps between the chain
  tanhs.  x update (stale by one step so the cast fires early):
      x_t = x_{t-1} + DT(A x_{t-1} + By y_{t-1} + Bw c_{t-1})

Schedule: ACT runs exactly [tanh1, wx', tanh2, wxx', tanh3] per step; the PE
queue is pinned via add_dep_helper in expected-fire-time order so no ready
instruction ever queues behind a stalled one (the previous kernel lost ~1us
per step to exactly that).
"""

import os
import sys

import numpy as np

for _p in ("/opt/trn_rl_repo", os.path.dirname(os.path.abspath(__file__))):
    if _p not in sys.path:
        sys.path.insert(0, _p)

import ml_dtypes  # noqa: E402

import concourse.bass as bass  # noqa: E402
import concourse.tile as tile  # noqa: E402
from concourse import bacc, mybir  # noqa: E402
from concourse.tile_rust import add_dep_helper  # noqa: E402


F32 = mybir.dt.float32
BF16 = mybir.dt.bfloat16
AF = mybir.ActivationFunctionType

# Model dims
B_FULL, T_FULL = 1024, 1024
NY, NX, NW, NU = 32, 16, 128, 8
DT = 0.01
N_COLD = 30
LOG_STD_INIT = -1.6094379124341003

N_CORES = 8
B = B_FULL // N_CORES  # 128 batch columns per core
NP = 64   # [x(16); 0(16); y_t(32)]
NPE = 96  # [x(16); 0(16); y_t(32); y_{t+1}(32)]

U_STEPS = 32   # steps per loop body (two slab halves of 16)
N_BODIES = 32  # covers t = 1 .. 1024
SL_STEPS = U_STEPS // 2
T_PAD = 1 + N_BODIES * U_STEPS


def padstack(top, bot):
    cols = top.shape[1]
    return np.concatenate(
        [top, np.zeros((32 - NX, cols), np.float64), bot], axis=0)


def expansion_matrices(A_T, Bw_T, By_T, Cv_T, Dvw_T, Dvy_T, Cu_T, Duw_T,
                       Duy_T):
    f = np.float64
    A_T, Bw_T, By_T = f(A_T), f(Bw_T), f(By_T)
    Cv_T, Dvw_T, Dvy_T = f(Cv_T), f(Dvw_T), f(Dvy_T)
    Cu_T, Duw_T, Duy_T = f(Cu_T), f(Duw_T), f(Duy_T)

    # eb_{t} = x_t Cv + y_t Dvy expanded over [x_{t-1}; 0; y_{t-1}; y_t]
    # plus a (DT Bw Cv) @ c_{t-1} term, merged into the chain weight.
    bxyd = np.concatenate([
        padstack(Cv_T + DT * (A_T @ Cv_T), DT * (By_T @ Cv_T)),
        Dvy_T], axis=0)  # [96, NW]

    g = lambda m: np.asarray(m, np.float32)
    return dict(
        dvwb=g(Dvw_T + DT * (Bw_T @ Cv_T)),  # merged chain weight
        dvw=g(Dvw_T),                        # u-track refinements + cold solve
        bxyd=g(bxyd),                        # chain-slot bias seed
        cvdvy=g(padstack(Cv_T, Dvy_T)),      # exact bias from [x_t; y_t]
        exy=g(DT * padstack(A_T, By_T)),     # x increment (A, By part)
        ew=g(DT * Bw_T),                     # x increment (Bw part)
        cuduy=g(padstack(Cu_T, Duy_T)),      # action
        duw=g(Duw_T),
    )


# weight shapes ([in, out])
W_SHAPES = dict(
    dvwb=[NW, NW], dvw=[NW, NW], bxyd=[NPE, NW], cvdvy=[NP, NW],
    exy=[NP, NX], ew=[NW, NX], cuduy=[NP, NU], duw=[NW, NU])


def _bf(a):
    return np.asarray(a, dtype=ml_dtypes.bfloat16)


def build_program(n_bodies=N_BODIES, u_steps=U_STEPS, n_cold=N_COLD):
    """Build + compile the per-core SPMD program."""
    t_pad = 1 + n_bodies * u_steps
    nc = bacc.Bacc("TRN2", debug=False, enable_asserts=False,
                   num_devices=N_CORES)

    sl_steps = u_steps // 2
    n_blocks = 2 * n_bodies + 1  # +1 zero pad (prefetch overrun)
    obs_slab_d = nc.dram_tensor(
        "obs_slab", [n_blocks * NY, sl_steps * B], BF16,
        kind="ExternalInput").ap()
    obs0_d = nc.dram_tensor("obs0", [NY, B], BF16, kind="ExternalInput").ap()
    x0_d = nc.dram_tensor("x0t", [NX, B], F32, kind="ExternalInput").ap()
    eye_d = nc.dram_tensor("eye16", [NX, NX], F32, kind="ExternalInput").ap()
    wd = {k: nc.dram_tensor(f"w_{k}", shp, BF16, kind="ExternalInput").ap()
          for k, shp in W_SHAPES.items()}
    u_out_d = nc.dram_tensor("u_out", [t_pad * NU, B], F32,
                             kind="ExternalOutput").ap()

    with tile.TileContext(nc) as tc:
        _build_kernel(tc, obs_slab_d, obs0_d, x0_d, eye_d, wd, u_out_d,
                      n_bodies, u_steps, n_cold)

    nc.compile()
    return nc, t_pad


def _build_kernel(tc, obs_slab_d, obs0_d, x0_d, eye_d, wd, u_out_d,
                  n_bodies, u_steps, n_cold):
    nc = tc.nc
    from contextlib import ExitStack

    sl_steps = u_steps // 2

    with ExitStack() as ctx:
        wpool = ctx.enter_context(tc.tile_pool(name="wpool", bufs=1))
        state = ctx.enter_context(tc.tile_pool(name="state", bufs=1))
        ustagp = ctx.enter_context(tc.tile_pool(name="ustagp", bufs=3))
        psum = ctx.enter_context(tc.tile_pool(name="psum", bufs=1,
                                              space="PSUM"))

        w = {}
        for k, d in wd.items():
            w[k] = wpool.tile(list(d.shape), BF16, name=f"w_{k}_sb")
            nc.sync.dma_start(w[k][:], d)
        eye_sb = wpool.tile([NX, NX], F32, name="eye_sb")
        nc.sync.dma_start(eye_sb[:], eye_d)
        x0_sb = wpool.tile([NX, B], F32, name="x0_sb")
        nc.sync.dma_start(x0_sb[:], x0_d)

        # double-buffered per-step state (parity = (t-1) % 2)
        w1b = [state.tile([NW, B], BF16, name=f"w1_{p}") for p in range(2)]
        w2b = [state.tile([NW, B], BF16, name=f"w2_{p}") for p in range(2)]
        cb = [state.tile([NW, B], BF16, name=f"c_{p}") for p in range(2)]
        wxb = [state.tile([NW, B], BF16, name=f"wx_{p}") for p in range(2)]
        wxxb = [state.tile([NW, B], BF16, name=f"wxx_{p}") for p in range(2)]
        # xe duplicated to 2B columns so one wide matmul can seed two PSUM
        # slots sharing a bank
        xeb = [state.tile([NPE, 2 * B], BF16, name=f"xe_{p}")
               for p in range(2)]
        slabs = [state.tile([NY, sl_steps * B], BF16, name=f"slab{h}")
                 for h in range(2)]

        # PSUM: sAB packs chain slots A|B side by side in one bank (seeded
        # by ONE wide bxyd matmul); sXP packs the wx|wxx slots (one wide
        # cvdvy matmul).  Both double-buffered so reseeds never WAR against
        # the current step's tanh reads.  8 banks total.
        sAB = [psum.tile([NW, 2 * B], F32, name=f"sAB{p}") for p in range(2)]
        sXP = [psum.tile([NW, 2 * B], F32, name=f"sXP{p}") for p in range(2)]
        sC = psum.tile([NW, B], F32, name="sC")
        s_ps = psum.tile([NX, B], F32, name="s_ps")  # fp32 x accumulator
        ups = [psum.tile([NU, B], F32, name=f"ups{p}") for p in range(2)]

        def mm(out, lhsT, rhs, start, stop):
            return nc.tensor.matmul(out, lhsT, rhs, start=start, stop=stop,
                                    skip_group_check=True)

        # pinned same-engine total orders
        pe_prev = [None]
        act_prev = [None]

        def pmm(out, lhsT, rhs, start, stop, why=""):
            h = mm(out, lhsT, rhs, start, stop)
            if pe_prev[0] is not None:
                add_dep_helper(h.ins, pe_prev[0].ins, sync=False,
                               reason=why or "pe order")
            pe_prev[0] = h
            return h

        def pact(out, src, why=""):
            h = nc.scalar.activation(out, src, AF.Tanh)
            if act_prev[0] is not None:
                add_dep_helper(h.ins, act_prev[0].ins, sync=False,
                               reason=why or "act order")
            act_prev[0] = h
            return h

        def dup_copy(dst2, src):
            """Copy src [p, B] into dst2 [p, 2B] twice (broadcast read)."""
            p = src.shape[0]
            d = dst2.rearrange("p (r c) -> p r c", r=2)
            s = src.rearrange("p (r c) -> p r c", r=1).broadcast_to((p, 2, B))
            nc.vector.tensor_copy(d, s)

        # ================= prologue: t = 0 (cold solve) =================
        # xe_0 lives in xeb[1]: step u=0 (t=1) writes xe_1 into xeb[0] and
        # reads xe_0 from xeb[pp=1].
        nc.vector.memset(xeb[0][:], 0.0)
        nc.vector.memset(xeb[1][:], 0.0)
        nc.sync.dma_start(xeb[1][32:NP, 0:B], obs0_d)        # y_0
        nc.sync.dma_start(xeb[1][32:NP, B:2 * B], obs0_d)
        nc.sync.dma_start(slabs[0][:], obs_slab_d[0:NY, :])

        # x PSUM accumulator <- x0 (identity matmul, fp32)
        pmm(s_ps[:], eye_sb[:], x0_sb[:], True, False)
        dup_copy(xeb[1][0:NX, :], s_ps[:])                   # x_0
        dup_copy(xeb[1][NP:NPE, :], slabs[0][:, 0:B])        # y_1

        # cold solve: 30 iterations, result -> cb[1] (c_0; step u=0 has
        # pp=1).  Uses the single-width sC bank as scratch.
        nc.vector.memset(cb[1][:], 0.0)
        for i in range(n_cold):
            pmm(sC[:], w["cvdvy"][:], xeb[1][0:NP, 0:B], True, False)
            pmm(sC[:], w["dvw"][:], cb[1][:], False, True)
            pact(cb[1][:], sC[:])

        # prologue plants for step u=0 (t=1); sC is planted by the body.
        pmm(s_ps[:], w["exy"][:], xeb[1][0:NP, 0:B], False, False)  # x_1
        pmm(sXP[0][:], w["cvdvy"][:], xeb[1][0:NP, :], True, False)
        # eb_1 seed, first half only (u=0's head plants the second half)
        pmm(sAB[0][:, 0:B], w["bxyd"][:], xeb[1][:, 0:B], True, False)

        # ================= warm loop: t = ci*32 + u + 1 =================
        with tc.For_i(0, n_bodies, 1, staggered_reset=True,
                      hint_engines=(mybir.EngineType.PE,
                                    mybir.EngineType.Activation,
                                    mybir.EngineType.DVE,
                                    mybir.EngineType.SP)) as ci:
            pe_prev[0] = None
            act_prev[0] = None
            nc.sync.dma_start(
                slabs[1][:], obs_slab_d[bass.ds(ci * (2 * NY) + NY, NY), :])
            for u in range(u_steps):
                px, pp = u % 2, 1 - (u % 2)
                cP = cb[pp]
                half, off = divmod(u, sl_steps)
                h2, off2 = divmod(u + 1, sl_steps) if u < u_steps - 1 \
                    else (0, 0)
                yt = slabs[half][:, off * B:(off + 1) * B]
                yt1 = slabs[h2][:, off2 * B:(off2 + 1) * B]
                q = px  # sAB/sXP buffer used this step

                # --- head: gated on c_{t-1}, then gate-free fillers ---
                pmm(sAB[q][:, 0:B], w["dvwb"][:], cP[:], False, True)  # chain1
                pmm(sXP[q][:, 0:B], w["dvw"][:], cP[:], False, True)   # wx'
                pmm(s_ps[:], w["ew"][:], cP[:], False, False)  # x_t += Bw c
                # sC seed for THIS step's chain3 (from xe_{t-1})
                pmm(sC[:], w["bxyd"][:], xeb[pp][:, 0:B], True, False)
                # second half of this step's chain-bias seed: the head has
                # ~300ns slack before chain2's release; the tail does not
                pmm(sAB[q][:, B:2 * B], w["bxyd"][:], xeb[pp][:, B:2 * B],
                    True, False)

                # DVE: fill xe_t (both column halves)
                dup_copy(xeb[px][32:NP, :], yt)
                dup_copy(xeb[px][NP:NPE, :], yt1)
                dup_copy(xeb[px][0:NX, :], s_ps[:])          # x_t cast

                pact(w1b[px][:], sAB[q][:, 0:B], "tanh1")
                pact(wxb[pp][:], sXP[q][:, 0:B], "wx'")

                # --- mid: gated on tanh1 / wx' ---
                pmm(sAB[q][:, B:2 * B], w["dvwb"][:], w1b[px][:], False,
                    True)                                     # chain2
                pmm(sXP[q][:, B:2 * B], w["dvw"][:], wxb[pp][:], False,
                    True)                                     # wxx'
                # wide reseed of the OTHER sXP buffer (no WAR: last readers
                # finished a step ago)
                pmm(sXP[pp][:], w["cvdvy"][:], xeb[px][0:NP, :], True, False)

                pact(w2b[px][:], sAB[q][:, B:2 * B], "tanh2")
                pact(wxxb[pp][:], sXP[q][:, B:2 * B], "wxx'")

                # --- tail: gated on tanh2 / wxx' ---
                pmm(sC[:], w["dvwb"][:], w2b[px][:], False, True)   # chain3
                pmm(sAB[pp][:, 0:B], w["bxyd"][:], xeb[px][:, 0:B], True,
                    False)
                pmm(s_ps[:], w["exy"][:], xeb[px][0:NP, 0:B], False, False)
                pmm(ups[pp][:], w["cuduy"][:], xeb[pp][0:NP, 0:B], True,
                    False)
                pmm(ups[pp][:], w["duw"][:], wxxb[pp][:], False, True)

                pact(cb[px][:], sC[:], "tanh3")

                # u_{t-1} stages through ACT-identity in the post-tanh3 gap
                # (not DVE: a late copy on the in-order DVE queue stalled the
                # x-cast consumers before; GpSimd cannot read PSUM)
                ustag = ustagp.tile([NU, B], F32, tag="ustag", name="ustag")
                h = nc.scalar.activation(ustag[:], ups[pp][:], AF.Copy)
                add_dep_helper(h.ins, act_prev[0].ins, sync=False,
                               reason="act order")
                act_prev[0] = h
                nc.sync.dma_start(
                    u_out_d[bass.ds(ci * (u_steps * NU) + u * NU, NU), :],
                    ustag[:])

                if u == sl_steps - 1:
                    nc.sync.dma_start(
                        slabs[0][:],
                        obs_slab_d[bass.ds(ci * (2 * NY) + 2 * NY, NY), :])


def prepare_inputs(obs, x0, A_T, Bw_T, By_T, Cv_T, Dvw_T, Dvy_T, Cu_T,
                   Duw_T, Duy_T, n_bodies=N_BODIES, u_steps=U_STEPS):
    """Host-side shard + transpose + bf16 conversion + expansion."""
    T = obs.shape[1]
    sl_steps = u_steps // 2
    n_blocks = 2 * n_bodies + 1  # +1 zero pad
    t_slab = n_blocks * sl_steps
    M = expansion_matrices(A_T, Bw_T, By_T, Cv_T, Dvw_T, Dvy_T, Cu_T, Duw_T,
                           Duy_T)
    shared = {f"w_{k}": _bf(v) for k, v in M.items()}
    shared["eye16"] = np.eye(NX, dtype=np.float32)

    in_maps = []
    for c in range(N_CORES):
        bsl = slice(c * B, (c + 1) * B)
        obs_c = np.ascontiguousarray(obs[bsl].transpose(1, 2, 0))  # [T,NY,B]
        obs_pad = np.zeros((1 + t_slab, NY, B), np.float32)
        obs_pad[:T] = obs_c
        slab = obs_pad[1:1 + t_slab]
        slab = slab.reshape(n_blocks, sl_steps, NY, B)
        slab = slab.transpose(0, 2, 1, 3).reshape(n_blocks * NY,
                                                  sl_steps * B)
        in_maps.append(dict(
            obs_slab=_bf(slab),
            obs0=_bf(obs_pad[0]),
            x0t=np.ascontiguousarray(x0[bsl].T).astype(np.float32),
            **shared))
    return in_maps


def assemble_output(results, log_stds, t_pad=T_PAD):
    out = np.empty((B_FULL, T_FULL, 2 * NU), np.float32)
    for c, res in enumerate(results):
        u = res["u_out"].reshape(t_pad, NU, B)[:T_FULL]
        out[c * B:(c + 1) * B, :, :NU] = u.transpose(2, 0, 1)
    out[:, :, NU:] = np.asarray(log_stds, np.float32)
    return out


_CACHE = {}


def _get_program():
    if "nc" not in _CACHE:
        _CACHE["nc"] = build_program()
    return _CACHE["nc"]


def kernel(obs, x0, A_T, Bw_T, By_T, Cv_T, Dvw_T, Dvy_T, Cu_T, Duw_T, Duy_T,
           log_stds):
    from concourse.bass_utils import run_bass_kernel_spmd

    nc, t_pad = _get_program()
    in_maps = prepare_inputs(obs, x0, A_T, Bw_T, By_T, Cv_T, Dvw_T, Dvy_T,
                             Cu_T, Duw_T, Duy_T)
    trace = bool(int(os.environ.get("RINN_TRACE", "0")))
    res = run_bass_kernel_spmd(nc, in_maps, core_ids=list(range(N_CORES)),
                               trace=trace)
    if trace:
        _CACHE["last_results"] = res
    return assemble_output(res.results, log_stds, t_pad)


# revision 24
# speedup vs baseline: 1.1029x; 1.1029x over previous
"""Trainium2 Bass kernel for DissipativeSimplestRINN.

Recurrent implicit NN: per time step, a warm-started tanh fixed-point solve
feeds an explicit-Euler integration of a small linear plant.  B=1024 batch is
sharded 8 ways (128/core); each core runs its batch slice through all T=1024
steps.

Numerical scheme (validated against the full reference in numpy/bf16 at
rel err 5.7e-3; gate is 2e-2):
  carry chain (3 links/step, on the critical path):
      w1 = tanh(eb_t + DVWB c_{t-1});  w2 = tanh(eb_t + DVWB w1);
      c_t = tanh(eb_t + DVWB w2)
  where DVWB = Dvw + DT*Bw@Cv and eb_t is the bias expansion over
  [x_{t-1}; y_{t-1}; y_t] (so the chain never waits on the x update).
  u-track (2 extra refinements, exact bias, off the critical path):
      wx_t = tanh(bx_t + Dvw c_t);  wxx_t = tanh(bx_t + Dvw wx_t)
      u_t = Cu x_t + Duw wxx_t + Duy y_t
  with bx_t = Cv x_t + Dvy y_t from the fp32 PSUM x accumulator.  wx/wxx of
  step t execute during step t+1, filling the ACT gaps between the chain
  tanhs.  x update (stale by one step so the cast fires early):
      x_t = x_{t-1} + DT(A x_{t-1} + By y_{t-1} + Bw c_{t-1})

Schedule: ACT runs exactly [tanh1, wx', tanh2, wxx', tanh3] per step; the PE
queue is pinned via add_dep_helper in expected-fire-time order so no ready
instruction ever queues behind a stalled one (the previous kernel lost ~1us
per step to exactly that).
"""

import os
import sys

import numpy as np

for _p in ("/opt/trn_rl_repo", os.path.dirname(os.path.abspath(__file__))):
    if _p not in sys.path:
        sys.path.insert(0, _p)

import ml_dtypes  # noqa: E402

import concourse.bass as bass  # noqa: E402
import concourse.tile as tile  # noqa: E402
from concourse import bacc, mybir  # noqa: E402
from concourse.tile_rust import add_dep_helper  # noqa: E402


F32 = mybir.dt.float32
BF16 = mybir.dt.bfloat16
AF = mybir.ActivationFunctionType

# Model dims
B_FULL, T_FULL = 1024, 1024
NY, NX, NW, NU = 32, 16, 128, 8
DT = 0.01
N_COLD = 30
LOG_STD_INIT = -1.6094379124341003

N_CORES = 8
B = B_FULL // N_CORES  # 128 batch columns per core
NP = 64   # [x(16); 0(16); y_t(32)]
NPE = 96  # [x(16); 0(16); y_t(32); y_{t+1}(32)]

U_STEPS = 32   # steps per loop body (two slab halves of 16)
N_BODIES = 32  # covers t = 1 .. 1024
SL_STEPS = U_STEPS // 2
T_PAD = 1 + N_BODIES * U_STEPS


def padstack(top, bot):
    cols = top.shape[1]
    return np.concatenate(
        [top, np.zeros((32 - NX, cols), np.float64), bot], axis=0)


def expansion_matrices(A_T, Bw_T, By_T, Cv_T, Dvw_T, Dvy_T, Cu_T, Duw_T,
                       Duy_T):
    f = np.float64
    A_T, Bw_T, By_T = f(A_T), f(Bw_T), f(By_T)
    Cv_T, Dvw_T, Dvy_T = f(Cv_T), f(Dvw_T), f(Dvy_T)
    Cu_T, Duw_T, Duy_T = f(Cu_T), f(Duw_T), f(Duy_T)

    # eb_{t} = x_t Cv + y_t Dvy expanded over [x_{t-1}; 0; y_{t-1}; y_t]
    # plus a (DT Bw Cv) @ c_{t-1} term, merged into the chain weight.
    bxyd = np.concatenate([
        padstack(Cv_T + DT * (A_T @ Cv_T), DT * (By_T @ Cv_T)),
        Dvy_T], axis=0)  # [96, NW]

    g = lambda m: np.asarray(m, np.float32)
    return dict(
        dvwb=g(Dvw_T + DT * (Bw_T @ Cv_T)),  # merged chain weight
        dvw=g(Dvw_T),                        # u-track refinements + cold solve
        bxyd=g(bxyd),                        # chain-slot bias seed
        cvdvy=g(padstack(Cv_T, Dvy_T)),      # exact bias from [x_t; y_t]
        exy=g(DT * padstack(A_T, By_T)),     # x increment (A, By part)
        ew=g(DT * Bw_T),                     # x increment (Bw part)
        cuduy=g(padstack(Cu_T, Duy_T)),      # action
        duw=g(Duw_T),
    )


# weight shapes ([in, out])
W_SHAPES = dict(
    dvwb=[NW, NW], dvw=[NW, NW], bxyd=[NPE, NW], cvdvy=[NP, NW],
    exy=[NP, NX], ew=[NW, NX], cuduy=[NP, NU], duw=[NW, NU])


def _bf(a):
    return np.asarray(a, dtype=ml_dtypes.bfloat16)


def build_program(n_bodies=N_BODIES, u_steps=U_STEPS, n_cold=N_COLD):
    """Build + compile the per-core SPMD program."""
    t_pad = 1 + n_bodies * u_steps
    nc = bacc.Bacc("TRN2", debug=False, enable_asserts=False,
                   num_devices=N_CORES)

    sl_steps = u_steps // 2
    n_blocks = 2 * n_bodies + 1  # +1 zero pad (prefetch overrun)
    obs_slab_d = nc.dram_tensor(
        "obs_slab", [n_blocks * NY, sl_steps * B], BF16,
        kind="ExternalInput").ap()
    obs0_d = nc.dram_tensor("obs0", [NY, B], BF16, kind="ExternalInput").ap()
    x0_d = nc.dram_tensor("x0t", [NX, B], F32, kind="ExternalInput").ap()
    eye_d = nc.dram_tensor("eye16", [NX, NX], F32, kind="ExternalInput").ap()
    wd = {k: nc.dram_tensor(f"w_{k}", shp, BF16, kind="ExternalInput").ap()
          for k, shp in W_SHAPES.items()}
    u_out_d = nc.dram_tensor("u_out", [t_pad * NU, B], F32,
                             kind="ExternalOutput").ap()

    with tile.TileContext(nc) as tc:
        _build_kernel(tc, obs_slab_d, obs0_d, x0_d, eye_d, wd, u_out_d,
                      n_bodies, u_steps, n_cold)

    nc.compile()
    return nc, t_pad


def _build_kernel(tc, obs_slab_d, obs0_d, x0_d, eye_d, wd, u_out_d,
                  n_bodies, u_steps, n_cold):
    nc = tc.nc
    from contextlib import ExitStack

    sl_steps = u_steps // 2

    with ExitStack() as ctx:
        wpool = ctx.enter_context(tc.tile_pool(name="wpool", bufs=1))
        state = ctx.enter_context(tc.tile_pool(name="state", bufs=1))
        ustagp = ctx.enter_context(tc.tile_pool(name="ustagp", bufs=3))
        psum = ctx.enter_context(tc.tile_pool(name="psum", bufs=1,
                                              space="PSUM"))

        w = {}
        for k, d in wd.items():
            w[k] = wpool.tile(list(d.shape), BF16, name=f"w_{k}_sb")
            nc.sync.dma_start(w[k][:], d)
        eye_sb = wpool.tile([NX, NX], F32, name="eye_sb")
        nc.sync.dma_start(eye_sb[:], eye_d)
        x0_sb = wpool.tile([NX, B], F32, name="x0_sb")
        nc.sync.dma_start(x0_sb[:], x0_d)

        # double-buffered per-step state (parity = (t-1) % 2)
        w1b = [state.tile([NW, B], BF16, name=f"w1_{p}") for p in range(2)]
        w2b = [state.tile([NW, B], BF16, name=f"w2_{p}") for p in range(2)]
        cb = [state.tile([NW, B], BF16, name=f"c_{p}") for p in range(2)]
        wxb = [state.tile([NW, B], BF16, name=f"wx_{p}") for p in range(2)]
        wxxb = [state.tile([NW, B], BF16, name=f"wxx_{p}") for p in range(2)]
        # xe duplicated to 2B columns so one wide matmul can seed two PSUM
        # slots sharing a bank
        xeb = [state.tile([NPE, 2 * B], BF16, name=f"xe_{p}")
               for p in range(2)]
        slabs = [state.tile([NY, sl_steps * B], BF16, name=f"slab{h}")
                 for h in range(2)]

        # PSUM: sAB packs chain slots A|B side by side in one bank (seeded
        # by ONE wide bxyd matmul); sXP packs the wx|wxx slots (one wide
        # cvdvy matmul).  Both double-buffered so reseeds never WAR against
        # the current step's tanh reads.  8 banks total.
        sAB = [psum.tile([NW, 2 * B], F32, name=f"sAB{p}") for p in range(2)]
        sXP = [psum.tile([NW, 2 * B], F32, name=f"sXP{p}") for p in range(2)]
        sC = psum.tile([NW, B], F32, name="sC")
        s_ps = psum.tile([NX, B], F32, name="s_ps")  # fp32 x accumulator
        ups = [psum.tile([NU, B], F32, name=f"ups{p}") for p in range(2)]

        def mm(out, lhsT, rhs, start, stop):
            return nc.tensor.matmul(out, lhsT, rhs, start=start, stop=stop,
                                    skip_group_check=True)

        # pinned same-engine total orders
        pe_prev = [None]
        act_prev = [None]

        def pmm(out, lhsT, rhs, start, stop, why=""):
            h = mm(out, lhsT, rhs, start, stop)
            if pe_prev[0] is not None:
                add_dep_helper(h.ins, pe_prev[0].ins, sync=False,
                               reason=why or "pe order")
            pe_prev[0] = h
            return h

        def pact(out, src, why=""):
            h = nc.scalar.activation(out, src, AF.Tanh)
            if act_prev[0] is not None:
                add_dep_helper(h.ins, act_prev[0].ins, sync=False,
                               reason=why or "act order")
            act_prev[0] = h
            return h

        def dup_copy(dst2, src):
            """Copy src [p, B] into dst2 [p, 2B] twice (broadcast read)."""
            p = src.shape[0]
            d = dst2.rearrange("p (r c) -> p r c", r=2)
            s = src.rearrange("p (r c) -> p r c", r=1).broadcast_to((p, 2, B))
            nc.vector.tensor_copy(d, s)

        # ================= prologue: t = 0 (cold solve) =================
        # xe_0 lives in xeb[1]: step u=0 (t=1) writes xe_1 into xeb[0] and
        # reads xe_0 from xeb[pp=1].
        nc.vector.memset(xeb[0][:], 0.0)
        nc.vector.memset(xeb[1][:], 0.0)
        nc.sync.dma_start(xeb[1][32:NP, 0:B], obs0_d)        # y_0
        nc.sync.dma_start(xeb[1][32:NP, B:2 * B], obs0_d)
        nc.sync.dma_start(slabs[0][:], obs_slab_d[0:NY, :])

        # x PSUM accumulator <- x0 (identity matmul, fp32)
        pmm(s_ps[:], eye_sb[:], x0_sb[:], True, False)
        dup_copy(xeb[1][0:NX, :], s_ps[:])                   # x_0
        dup_copy(xeb[1][NP:NPE, :], slabs[0][:, 0:B])        # y_1

        # cold solve: 30 iterations, result -> cb[1] (c_0; step u=0 has
        # pp=1).  Uses the single-width sC bank as scratch.
        nc.vector.memset(cb[1][:], 0.0)
        for i in range(n_cold):
            pmm(sC[:], w["cvdvy"][:], xeb[1][0:NP, 0:B], True, False)
            pmm(sC[:], w["dvw"][:], cb[1][:], False, True)
            pact(cb[1][:], sC[:])

        # prologue plants for step u=0 (t=1); sC is planted by the body.
        pmm(s_ps[:], w["exy"][:], xeb[1][0:NP, 0:B], False, False)  # x_1
        pmm(sXP[0][:], w["cvdvy"][:], xeb[1][0:NP, :], True, False)
        pmm(sAB[0][:], w["bxyd"][:], xeb[1][:], True, False)  # eb_1 seeds

        # ================= warm loop: t = ci*32 + u + 1 =================
        with tc.For_i(0, n_bodies, 1, staggered_reset=True,
                      hint_engines=(mybir.EngineType.PE,
                                    mybir.EngineType.Activation,
                                    mybir.EngineType.DVE,
                                    mybir.EngineType.SP)) as ci:
            pe_prev[0] = None
            act_prev[0] = None
            nc.sync.dma_start(
                slabs[1][:], obs_slab_d[bass.ds(ci * (2 * NY) + NY, NY), :])
            for u in range(u_steps):
                px, pp = u % 2, 1 - (u % 2)
                cP = cb[pp]
                half, off = divmod(u, sl_steps)
                h2, off2 = divmod(u + 1, sl_steps) if u < u_steps - 1 \
                    else (0, 0)
                yt = slabs[half][:, off * B:(off + 1) * B]
                yt1 = slabs[h2][:, off2 * B:(off2 + 1) * B]
                q = px  # sAB/sXP buffer used this step

                # --- head: gated on c_{t-1}, then gate-free fillers ---
                pmm(sAB[q][:, 0:B], w["dvwb"][:], cP[:], False, True)  # chain1
                pmm(sXP[q][:, 0:B], w["dvw"][:], cP[:], False, True)   # wx'
                pmm(s_ps[:], w["ew"][:], cP[:], False, False)  # x_t += Bw c
                # sC seed for THIS step's chain3 (from xe_{t-1})
                pmm(sC[:], w["bxyd"][:], xeb[pp][:, 0:B], True, False)

                # DVE: fill xe_t (both column halves)
                dup_copy(xeb[px][32:NP, :], yt)
                dup_copy(xeb[px][NP:NPE, :], yt1)
                dup_copy(xeb[px][0:NX, :], s_ps[:])          # x_t cast

                pact(w1b[px][:], sAB[q][:, 0:B], "tanh1")
                pact(wxb[pp][:], sXP[q][:, 0:B], "wx'")

                # --- mid: gated on tanh1 / wx' ---
                pmm(sAB[q][:, B:2 * B], w["dvwb"][:], w1b[px][:], False,
                    True)                                     # chain2
                pmm(sXP[q][:, B:2 * B], w["dvw"][:], wxb[pp][:], False,
                    True)                                     # wxx'
                # wide reseed of the OTHER sXP buffer (no WAR: last readers
                # finished a step ago)
                pmm(sXP[pp][:], w["cvdvy"][:], xeb[px][0:NP, :], True, False)

                pact(w2b[px][:], sAB[q][:, B:2 * B], "tanh2")
                pact(wxxb[pp][:], sXP[q][:, B:2 * B], "wxx'")

                # --- tail: gated on tanh2 / wxx' ---
                pmm(sC[:], w["dvwb"][:], w2b[px][:], False, True)   # chain3
                # u first: its wxx' gate cleared long ago, and finishing it
                # early lets the ACT u-copy fill the tanh3->tanh1 gap
                pmm(ups[pp][:], w["cuduy"][:], xeb[pp][0:NP, 0:B], True,
                    False)
                pmm(ups[pp][:], w["duw"][:], wxxb[pp][:], False, True)
                pmm(sAB[pp][:], w["bxyd"][:], xeb[px][:], True, False)
                pmm(s_ps[:], w["exy"][:], xeb[px][0:NP, 0:B], False, False)

                pact(cb[px][:], sC[:], "tanh3")

                # u_{t-1} stages through ACT-identity in the post-tanh3 gap
                # (not DVE: a late copy on the in-order DVE queue stalled the
                # x-cast consumers before; GpSimd cannot read PSUM)
                ustag = ustagp.tile([NU, B], F32, tag="ustag", name="ustag")
                h = nc.scalar.activation(ustag[:], ups[pp][:], AF.Copy)
                add_dep_helper(h.ins, act_prev[0].ins, sync=False,
                               reason="act order")
                act_prev[0] = h
                nc.sync.dma_start(
                    u_out_d[bass.ds(ci * (u_steps * NU) + u * NU, NU), :],
                    ustag[:])

                if u == sl_steps - 1:
                    nc.sync.dma_start(
                        slabs[0][:],
                        obs_slab_d[bass.ds(ci * (2 * NY) + 2 * NY, NY), :])


def prepare_inputs(obs, x0, A_T, Bw_T, By_T, Cv_T, Dvw_T, Dvy_T, Cu_T,
                   Duw_T, Duy_T, n_bodies=N_BODIES, u_steps=U_STEPS):
    """Host-side shard + transpose + bf16 conversion + expansion."""
    T = obs.shape[1]
    sl_steps = u_steps // 2
    n_blocks = 2 * n_bodies + 1  # +1 zero pad
    t_slab = n_blocks * sl_steps
    M = expansion_matrices(A_T, Bw_T, By_T, Cv_T, Dvw_T, Dvy_T, Cu_T, Duw_T,
                           Duy_T)
    shared = {f"w_{k}": _bf(v) for k, v in M.items()}
    shared["eye16"] = np.eye(NX, dtype=np.float32)

    in_maps = []
    for c in range(N_CORES):
        bsl = slice(c * B, (c + 1) * B)
        obs_c = np.ascontiguousarray(obs[bsl].transpose(1, 2, 0))  # [T,NY,B]
        obs_pad = np.zeros((1 + t_slab, NY, B), np.float32)
        obs_pad[:T] = obs_c
        slab = obs_pad[1:1 + t_slab]
        slab = slab.reshape(n_blocks, sl_steps, NY, B)
        slab = slab.transpose(0, 2, 1, 3).reshape(n_blocks * NY,
                                                  sl_steps * B)
        in_maps.append(dict(
            obs_slab=_bf(slab),
            obs0=_bf(obs_pad[0]),
            x0t=np.ascontiguousarray(x0[bsl].T).astype(np.float32),
            **shared))
    return in_maps


def assemble_output(results, log_stds, t_pad=T_PAD):
    out = np.empty((B_FULL, T_FULL, 2 * NU), np.float32)
    for c, res in enumerate(results):
        u = res["u_out"].reshape(t_pad, NU, B)[:T_FULL]
        out[c * B:(c + 1) * B, :, :NU] = u.transpose(2, 0, 1)
    out[:, :, NU:] = np.asarray(log_stds, np.float32)
    return out


_CACHE = {}


def _get_program():
    if "nc" not in _CACHE:
        _CACHE["nc"] = build_program()
    return _CACHE["nc"]


def kernel(obs, x0, A_T, Bw_T, By_T, Cv_T, Dvw_T, Dvy_T, Cu_T, Duw_T, Duy_T,
           log_stds):
    from concourse.bass_utils import run_bass_kernel_spmd

    nc, t_pad = _get_program()
    in_maps = prepare_inputs(obs, x0, A_T, Bw_T, By_T, Cv_T, Dvw_T, Dvy_T,
                             Cu_T, Duw_T, Duy_T)
    trace = bool(int(os.environ.get("RINN_TRACE", "0")))
    res = run_bass_kernel_spmd(nc, in_maps, core_ids=list(range(N_CORES)),
                               trace=trace)
    if trace:
        _CACHE["last_results"] = res
    return assemble_output(res.results, log_stds, t_pad)


# revision 25
# speedup vs baseline: 1.1707x; 1.0614x over previous
"""Trainium2 Bass kernel for DissipativeSimplestRINN.

Recurrent implicit NN: per time step, a warm-started tanh fixed-point solve
feeds an explicit-Euler integration of a small linear plant.  B=1024 batch is
sharded 8 ways (128/core); each core runs its batch slice through all T=1024
steps.

Numerical scheme (validated against the full reference in numpy/bf16 at
rel err 5.7e-3; gate is 2e-2):
  carry chain (3 links/step, on the critical path):
      w1 = tanh(eb_t + DVWB c_{t-1});  w2 = tanh(eb_t + DVWB w1);
      c_t = tanh(eb_t + DVWB w2)
  where DVWB = Dvw + DT*Bw@Cv and eb_t is the bias expansion over
  [x_{t-1}; y_{t-1}; y_t] (so the chain never waits on the x update).
  u-track (2 extra refinements, exact bias, off the critical path):
      wx_t = tanh(bx_t + Dvw c_t);  wxx_t = tanh(bx_t + Dvw wx_t)
      u_t = Cu x_t + Duw wxx_t + Duy y_t
  with bx_t = Cv x_t + Dvy y_t from the fp32 PSUM x accumulator.  wx/wxx of
  step t execute during step t+1, filling the ACT gaps between the chain
  tanhs.  x update (stale by one step so the cast fires early):
      x_t = x_{t-1} + DT(A x_{t-1} + By y_{t-1} + Bw c_{t-1})

Schedule: ACT runs exactly [tanh1, wx', tanh2, wxx', tanh3] per step; the PE
queue is pinned via add_dep_helper in expected-fire-time order so no ready
instruction ever queues behind a stalled one (the previous kernel lost ~1us
per step to exactly that).
"""

import os
import sys

import numpy as np

for _p in ("/opt/trn_rl_repo", os.path.dirname(os.path.abspath(__file__))):
    if _p not in sys.path:
        sys.path.insert(0, _p)

import ml_dtypes  # noqa: E402

import concourse.bass as bass  # noqa: E402
import concourse.tile as tile  # noqa: E402
from concourse import bacc, mybir  # noqa: E402
from concourse.tile_rust import add_dep_helper  # noqa: E402


F32 = mybir.dt.float32
BF16 = mybir.dt.bfloat16
AF = mybir.ActivationFunctionType

# Model dims
B_FULL, T_FULL = 1024, 1024
NY, NX, NW, NU = 32, 16, 128, 8
DT = 0.01
N_COLD = 30
LOG_STD_INIT = -1.6094379124341003

N_CORES = 8
B = B_FULL // N_CORES  # 128 batch columns per core
NP = 64   # [x(16); 0(16); y_t(32)]
NPE = 96  # [x(16); 0(16); y_t(32); y_{t+1}(32)]

U_STEPS = 32   # steps per loop body (two slab halves of 16)
N_BODIES = 32  # covers t = 1 .. 1024
SL_STEPS = U_STEPS // 2
T_PAD = 1 + N_BODIES * U_STEPS


def padstack(top, bot):
    cols = top.shape[1]
    return np.concatenate(
        [top, np.zeros((32 - NX, cols), np.float64), bot], axis=0)


def expansion_matrices(A_T, Bw_T, By_T, Cv_T, Dvw_T, Dvy_T, Cu_T, Duw_T,
                       Duy_T):
    f = np.float64
    A_T, Bw_T, By_T = f(A_T), f(Bw_T), f(By_T)
    Cv_T, Dvw_T, Dvy_T = f(Cv_T), f(Dvw_T), f(Dvy_T)
    Cu_T, Duw_T, Duy_T = f(Cu_T), f(Duw_T), f(Duy_T)

    # eb_{t} = x_t Cv + y_t Dvy expanded over [x_{t-1}; 0; y_{t-1}; y_t]
    # plus a (DT Bw Cv) @ c_{t-1} term, merged into the chain weight.
    bxyd = np.concatenate([
        padstack(Cv_T + DT * (A_T @ Cv_T), DT * (By_T @ Cv_T)),
        Dvy_T], axis=0)  # [96, NW]

    g = lambda m: np.asarray(m, np.float32)
    return dict(
        dvwb=g(Dvw_T + DT * (Bw_T @ Cv_T)),  # merged chain weight
        dvw=g(Dvw_T),                        # u-track refinements + cold solve
        bxyd=g(bxyd),                        # chain-slot bias seed
        cvdvy=g(padstack(Cv_T, Dvy_T)),      # exact bias from [x_t; y_t]
        exy=g(DT * padstack(A_T, By_T)),     # x increment (A, By part)
        ew=g(DT * Bw_T),                     # x increment (Bw part)
        cuduy=g(padstack(Cu_T, Duy_T)),      # action
        duw=g(Duw_T),
    )


# weight shapes ([in, out])
W_SHAPES = dict(
    dvwb=[NW, NW], dvw=[NW, NW], bxyd=[NPE, NW], cvdvy=[NP, NW],
    exy=[NP, NX], ew=[NW, NX], cuduy=[NP, NU], duw=[NW, NU])


def _bf(a):
    return np.asarray(a, dtype=ml_dtypes.bfloat16)


def build_program(n_bodies=N_BODIES, u_steps=U_STEPS, n_cold=N_COLD):
    """Build + compile the per-core SPMD program."""
    t_pad = 1 + n_bodies * u_steps
    nc = bacc.Bacc("TRN2", debug=False, enable_asserts=False,
                   num_devices=N_CORES)

    sl_steps = u_steps // 2
    n_blocks = 2 * n_bodies + 1  # +1 zero pad (prefetch overrun)
    obs_slab_d = nc.dram_tensor(
        "obs_slab", [n_blocks * NY, sl_steps * B], BF16,
        kind="ExternalInput").ap()
    obs0_d = nc.dram_tensor("obs0", [NY, B], BF16, kind="ExternalInput").ap()
    x0_d = nc.dram_tensor("x0t", [NX, B], F32, kind="ExternalInput").ap()
    eye_d = nc.dram_tensor("eye16", [NX, NX], F32, kind="ExternalInput").ap()
    wd = {k: nc.dram_tensor(f"w_{k}", shp, BF16, kind="ExternalInput").ap()
          for k, shp in W_SHAPES.items()}
    u_out_d = nc.dram_tensor("u_out", [t_pad * NU, B], F32,
                             kind="ExternalOutput").ap()

    with tile.TileContext(nc) as tc:
        _build_kernel(tc, obs_slab_d, obs0_d, x0_d, eye_d, wd, u_out_d,
                      n_bodies, u_steps, n_cold)

    nc.compile()
    return nc, t_pad


def _build_kernel(tc, obs_slab_d, obs0_d, x0_d, eye_d, wd, u_out_d,
                  n_bodies, u_steps, n_cold):
    nc = tc.nc
    from contextlib import ExitStack

    sl_steps = u_steps // 2

    with ExitStack() as ctx:
        wpool = ctx.enter_context(tc.tile_pool(name="wpool", bufs=1))
        state = ctx.enter_context(tc.tile_pool(name="state", bufs=1))
        ustagp = ctx.enter_context(tc.tile_pool(name="ustagp", bufs=3))
        psum = ctx.enter_context(tc.tile_pool(name="psum", bufs=1,
                                              space="PSUM"))

        w = {}
        for k, d in wd.items():
            w[k] = wpool.tile(list(d.shape), BF16, name=f"w_{k}_sb")
            nc.sync.dma_start(w[k][:], d)
        eye_sb = wpool.tile([NX, NX], F32, name="eye_sb")
        nc.sync.dma_start(eye_sb[:], eye_d)
        x0_sb = wpool.tile([NX, B], F32, name="x0_sb")
        nc.sync.dma_start(x0_sb[:], x0_d)

        # double-buffered per-step state (parity = (t-1) % 2)
        w1b = [state.tile([NW, B], BF16, name=f"w1_{p}") for p in range(2)]
        w2b = [state.tile([NW, B], BF16, name=f"w2_{p}") for p in range(2)]
        cb = [state.tile([NW, B], BF16, name=f"c_{p}") for p in range(2)]
        wxb = [state.tile([NW, B], BF16, name=f"wx_{p}") for p in range(2)]
        wxxb = [state.tile([NW, B], BF16, name=f"wxx_{p}") for p in range(2)]
        # xe duplicated to 2B columns so one wide matmul can seed two PSUM
        # slots sharing a bank
        xeb = [state.tile([NPE, 2 * B], BF16, name=f"xe_{p}")
               for p in range(2)]
        slabs = [state.tile([NY, sl_steps * B], BF16, name=f"slab{h}")
                 for h in range(2)]

        # PSUM: sAB packs chain slots A|B side by side in one bank (seeded
        # by ONE wide bxyd matmul); sXP packs the wx|wxx slots (one wide
        # cvdvy matmul).  Both double-buffered so reseeds never WAR against
        # the current step's tanh reads.  8 banks total.
        sAB = [psum.tile([NW, 2 * B], F32, name=f"sAB{p}") for p in range(2)]
        sXP = [psum.tile([NW, 2 * B], F32, name=f"sXP{p}") for p in range(2)]
        sC = psum.tile([NW, B], F32, name="sC")
        s_ps = psum.tile([NX, B], F32, name="s_ps")  # fp32 x accumulator
        ups = [psum.tile([NU, B], F32, name=f"ups{p}") for p in range(2)]

        def mm(out, lhsT, rhs, start, stop):
            return nc.tensor.matmul(out, lhsT, rhs, start=start, stop=stop,
                                    skip_group_check=True)

        # pinned same-engine total orders
        pe_prev = [None]
        act_prev = [None]

        def pmm(out, lhsT, rhs, start, stop, why=""):
            h = mm(out, lhsT, rhs, start, stop)
            if pe_prev[0] is not None:
                add_dep_helper(h.ins, pe_prev[0].ins, sync=False,
                               reason=why or "pe order")
            pe_prev[0] = h
            return h

        def pact(out, src, why=""):
            h = nc.scalar.activation(out, src, AF.Tanh)
            if act_prev[0] is not None:
                add_dep_helper(h.ins, act_prev[0].ins, sync=False,
                               reason=why or "act order")
            act_prev[0] = h
            return h

        def dup_copy(dst2, src):
            """Copy src [p, B] into dst2 [p, 2B] twice (broadcast read)."""
            p = src.shape[0]
            d = dst2.rearrange("p (r c) -> p r c", r=2)
            s = src.rearrange("p (r c) -> p r c", r=1).broadcast_to((p, 2, B))
            nc.vector.tensor_copy(d, s)

        # ================= prologue: t = 0 (cold solve) =================
        # xe_0 lives in xeb[1]: step u=0 (t=1) writes xe_1 into xeb[0] and
        # reads xe_0 from xeb[pp=1].
        nc.vector.memset(xeb[0][:], 0.0)
        nc.vector.memset(xeb[1][:], 0.0)
        nc.sync.dma_start(xeb[1][32:NP, 0:B], obs0_d)        # y_0
        nc.sync.dma_start(xeb[1][32:NP, B:2 * B], obs0_d)
        nc.sync.dma_start(slabs[0][:], obs_slab_d[0:NY, :])

        # x PSUM accumulator <- x0 (identity matmul, fp32)
        pmm(s_ps[:], eye_sb[:], x0_sb[:], True, False)
        dup_copy(xeb[1][0:NX, :], s_ps[:])                   # x_0
        dup_copy(xeb[1][NP:NPE, :], slabs[0][:, 0:B])        # y_1

        # cold solve: 30 iterations, result -> cb[1] (c_0; step u=0 has
        # pp=1).  Uses the single-width sC bank as scratch.
        nc.vector.memset(cb[1][:], 0.0)
        for i in range(n_cold):
            pmm(sC[:], w["cvdvy"][:], xeb[1][0:NP, 0:B], True, False)
            pmm(sC[:], w["dvw"][:], cb[1][:], False, True)
            pact(cb[1][:], sC[:])

        # prologue plants for step u=0 (t=1); sC is planted by the body.
        pmm(s_ps[:], w["exy"][:], xeb[1][0:NP, 0:B], False, False)  # x_1
        pmm(sXP[0][:], w["cvdvy"][:], xeb[1][0:NP, :], True, False)
        pmm(sAB[0][:], w["bxyd"][:], xeb[1][:], True, False)  # eb_1 seeds

        # ================= warm loop: t = ci*32 + u + 1 =================
        with tc.For_i(0, n_bodies, 1, staggered_reset=True,
                      hint_engines=(mybir.EngineType.PE,
                                    mybir.EngineType.Activation,
                                    mybir.EngineType.DVE,
                                    mybir.EngineType.SP)) as ci:
            pe_prev[0] = None
            act_prev[0] = None
            nc.sync.dma_start(
                slabs[1][:], obs_slab_d[bass.ds(ci * (2 * NY) + NY, NY), :])
            for u in range(u_steps):
                px, pp = u % 2, 1 - (u % 2)
                cP = cb[pp]
                half, off = divmod(u, sl_steps)
                h2, off2 = divmod(u + 1, sl_steps) if u < u_steps - 1 \
                    else (0, 0)
                yt = slabs[half][:, off * B:(off + 1) * B]
                yt1 = slabs[h2][:, off2 * B:(off2 + 1) * B]
                q = px  # sAB/sXP buffer used this step

                # --- head: gated on c_{t-1}, then gate-free fillers ---
                pmm(sAB[q][:, 0:B], w["dvwb"][:], cP[:], False, True)  # chain1
                pmm(sXP[q][:, 0:B], w["dvw"][:], cP[:], False, True)   # wx'
                pmm(s_ps[:], w["ew"][:], cP[:], False, False)  # x_t += Bw c
                # sC seed for THIS step's chain3 (from xe_{t-1})
                pmm(sC[:], w["bxyd"][:], xeb[pp][:, 0:B], True, False)

                # DVE: fill xe_t (both column halves)
                dup_copy(xeb[px][32:NP, :], yt)
                dup_copy(xeb[px][NP:NPE, :], yt1)
                dup_copy(xeb[px][0:NX, :], s_ps[:])          # x_t cast

                pact(w1b[px][:], sAB[q][:, 0:B], "tanh1")
                pact(wxb[pp][:], sXP[q][:, 0:B], "wx'")

                # --- mid: gated on tanh1 / wx' ---
                pmm(sAB[q][:, B:2 * B], w["dvwb"][:], w1b[px][:], False,
                    True)                                     # chain2
                pmm(sXP[q][:, B:2 * B], w["dvw"][:], wxb[pp][:], False,
                    True)                                     # wxx'
                # wide reseed of the OTHER sXP buffer (no WAR: last readers
                # finished a step ago)
                pmm(sXP[pp][:], w["cvdvy"][:], xeb[px][0:NP, :], True, False)

                pact(w2b[px][:], sAB[q][:, B:2 * B], "tanh2")
                pact(wxxb[pp][:], sXP[q][:, B:2 * B], "wxx'")

                # --- tail: gated on tanh2 / wxx' ---
                pmm(sC[:], w["dvwb"][:], w2b[px][:], False, True)   # chain3
                pmm(sAB[pp][:], w["bxyd"][:], xeb[px][:], True, False)
                pmm(s_ps[:], w["exy"][:], xeb[px][0:NP, 0:B], False, False)
                pmm(ups[pp][:], w["cuduy"][:], xeb[pp][0:NP, 0:B], True,
                    False)
                pmm(ups[pp][:], w["duw"][:], wxxb[pp][:], False, True)

                pact(cb[px][:], sC[:], "tanh3")

                # u_{t-1} stages through ACT-identity in the post-tanh3 gap
                # (not DVE: a late copy on the in-order DVE queue stalled the
                # x-cast consumers before; GpSimd cannot read PSUM)
                ustag = ustagp.tile([NU, B], F32, tag="ustag", name="ustag")
                h = nc.scalar.activation(ustag[:], ups[pp][:], AF.Copy)
                add_dep_helper(h.ins, act_prev[0].ins, sync=False,
                               reason="act order")
                act_prev[0] = h
                nc.sync.dma_start(
                    u_out_d[bass.ds(ci * (u_steps * NU) + u * NU, NU), :],
                    ustag[:])

                if u == sl_steps - 1:
                    nc.sync.dma_start(
                        slabs[0][:],
                        obs_slab_d[bass.ds(ci * (2 * NY) + 2 * NY, NY), :])


def prepare_inputs(obs, x0, A_T, Bw_T, By_T, Cv_T, Dvw_T, Dvy_T, Cu_T,
                   Duw_T, Duy_T, n_bodies=N_BODIES, u_steps=U_STEPS):
    """Host-side shard + transpose + bf16 conversion + expansion."""
    T = obs.shape[1]
    sl_steps = u_steps // 2
    n_blocks = 2 * n_bodies + 1  # +1 zero pad
    t_slab = n_blocks * sl_steps
    M = expansion_matrices(A_T, Bw_T, By_T, Cv_T, Dvw_T, Dvy_T, Cu_T, Duw_T,
                           Duy_T)
    shared = {f"w_{k}": _bf(v) for k, v in M.items()}
    shared["eye16"] = np.eye(NX, dtype=np.float32)

    in_maps = []
    for c in range(N_CORES):
        bsl = slice(c * B, (c + 1) * B)
        obs_c = np.ascontiguousarray(obs[bsl].transpose(1, 2, 0))  # [T,NY,B]
        obs_pad = np.zeros((1 + t_slab, NY, B), np.float32)
        obs_pad[:T] = obs_c
        slab = obs_pad[1:1 + t_slab]
        slab = slab.reshape(n_blocks, sl_steps, NY, B)
        slab = slab.transpose(0, 2, 1, 3).reshape(n_blocks * NY,
                                                  sl_steps * B)
        in_maps.append(dict(
            obs_slab=_bf(slab),
            obs0=_bf(obs_pad[0]),
            x0t=np.ascontiguousarray(x0[bsl].T).astype(np.float32),
            **shared))
    return in_maps


def assemble_output(results, log_stds, t_pad=T_PAD):
    out = np.empty((B_FULL, T_FULL, 2 * NU), np.float32)
    for c, res in enumerate(results):
        u = res["u_out"].reshape(t_pad, NU, B)[:T_FULL]
        out[c * B:(c + 1) * B, :, :NU] = u.transpose(2, 0, 1)
    out[:, :, NU:] = np.asarray(log_stds, np.float32)
    return out


_CACHE = {}


def _get_program():
    if "nc" not in _CACHE:
        _CACHE["nc"] = build_program()
    return _CACHE["nc"]


def kernel(obs, x0, A_T, Bw_T, By_T, Cv_T, Dvw_T, Dvy_T, Cu_T, Duw_T, Duy_T,
           log_stds):
    from concourse.bass_utils import run_bass_kernel_spmd

    nc, t_pad = _get_program()
    in_maps = prepare_inputs(obs, x0, A_T, Bw_T, By_T, Cv_T, Dvw_T, Dvy_T,
                             Cu_T, Duw_T, Duy_T)
    trace = bool(int(os.environ.get("RINN_TRACE", "0")))
    res = run_bass_kernel_spmd(nc, in_maps, core_ids=list(range(N_CORES)),
                               trace=trace)
    if trace:
        _CACHE["last_results"] = res
    return assemble_output(res.results, log_stds, t_pad)
